# revision 1
# baseline (speedup 1.0000x reference)
"""APPNP distributed kernel for 8 TRN2 NeuronCores.

Sharding: rows (nodes) split across 8 cores (degree-balanced snake deal).
Per propagation step: AllGather x -> x_full; per-core SpMM via
dma_gather (col-chunked int16 indices) + broadcast-multiply by edge val +
dma_scatter_add into a per-core accumulator, organized in "waves" so each
scatter instruction has unique destination rows (scatter-add loses updates
on duplicate destinations within one instruction).
"""
import sys
import types

import numpy as np

N_NODES = 100000
N_EDGES = 3200000
C = 8
JR = 98
R = 128 * JR          # 12544 rows/core (12500 real + 44 dead)
R_REAL = 12500
NTOT = R * C          # 100352
NCHUNK = 4
CHUNK = NTOT // NCHUNK  # 25088
TRASH = 128
ACC_ROWS = R + TRASH  # 12672
SUBC = 48             # max columns per block instruction
ALPHA = 0.1
K = 10


def _install_ntff_shim():
    if "antenv.axon_hooks" in sys.modules:
        return
    mod = types.ModuleType("antenv.axon_hooks")
    state = {}
    mod.set_axon_ntff_profile_hook = lambda h: state.__setitem__("h", h)
    mod.get_axon_ntff_profile_hook = lambda: state.get("h")
    sys.modules["antenv.axon_hooks"] = mod
    try:
        from trn_agent_boot.trn_boot import _ntff_profile_via_ctypes

        mod.set_axon_ntff_profile_hook(
            _ntff_profile_via_ctypes("/opt/axon/libaxon_pjrt.so")
        )
    except Exception:
        pass


_install_ntff_shim()

import concourse.bacc as bacc
import concourse.mybir as mybir
import concourse.tile as tile
from concourse import bass_utils


def rep_idx(flat_rows):
    """[n] or [C, n] int16 stream -> [(C,) 128, n//16*8] wrapped+replicated."""
    flat_rows = np.asarray(flat_rows, np.int16)
    single = flat_rows.ndim == 1
    if single:
        flat_rows = flat_rows[None]
    nc_, n = flat_rows.shape
    blk = flat_rows.reshape(nc_, n // 16, 16).transpose(0, 2, 1)  # [C,16,n/16]
    out = np.empty((nc_, 128, n // 16), np.int16)
    for k in range(8):
        out[:, 16 * k:16 * (k + 1), :] = blk
    return out[0] if single else out


def prep(features, W1, b1, W2, b2, edge_vals, edge_row, edge_col):
    """Host-side sharding. Returns (in_maps, blocks, TOT, order)."""
    edge_row = np.asarray(edge_row, np.int64)
    edge_col = np.asarray(edge_col, np.int64)
    deg = np.bincount(edge_row, minlength=N_NODES)
    order = np.argsort(-deg, kind="stable")      # rank -> global row
    rank = np.empty(N_NODES, np.int64)
    rank[order] = np.arange(N_NODES)
    core_of = rank % C
    local_of = rank // C
    newpos = core_of * R + local_of

    ec = core_of[edge_row]
    ed = local_of[edge_row]
    es = newpos[edge_col]
    ech = es // CHUNK
    ei = es % CHUNK

    # wave rank per edge within (core, chunk, dst)
    key = (ec * NCHUNK + ech) * R + ed
    o1 = np.argsort(key, kind="stable")
    ks = key[o1]
    new_grp = np.r_[True, ks[1:] != ks[:-1]]
    gstart = np.flatnonzero(new_grp)
    gid = np.cumsum(new_grp) - 1
    w = np.arange(ks.size) - gstart[gid]

    maxw = int(w.max()) + 1
    cnt = np.zeros((NCHUNK, maxw, C), np.int64)
    np.add.at(cnt, (ech[o1], w, ec[o1]), 1)
    nmax = cnt.max(axis=2)
    ncols = -(-nmax // 128)

    col_off = np.zeros((NCHUNK, maxw), np.int64)
    blocks = []
    tot = 0
    for c in range(NCHUNK):
        for wv in range(maxw):
            ncw = int(ncols[c, wv])
            if ncw == 0:
                continue
            col_off[c, wv] = tot
            s = 0
            while s < ncw:
                bc = min(SUBC, ncw - s)
                blocks.append((c, tot + s, bc))
                s += bc
            tot += ncw
    TOT = tot

    # position q within (core, chunk, wave), ordered by dst
    key2 = (ec[o1] * NCHUNK + ech[o1]) * maxw + w
    o2 = np.argsort(key2, kind="stable")
    k2 = key2[o2]
    ng2 = np.r_[True, k2[1:] != k2[:-1]]
    gs2 = np.flatnonzero(ng2)
    g2 = np.cumsum(ng2) - 1
    q = np.arange(k2.size) - gs2[g2]

    eidx = o1[o2]
    ecore = ec[eidx]
    slot = col_off[ech[eidx], w[o2]] * 128 + q

    gidx_flat = np.zeros((C, TOT * 128), np.int16)
    val_flat = np.zeros((C, TOT * 128), np.float32)
    pos = np.arange(TOT * 128)
    sdst_flat = np.broadcast_to(
        (R + (pos % 128)).astype(np.int16), (C, TOT * 128)
    ).copy()
    gidx_flat[ecore, slot] = ei[eidx].astype(np.int16)
    val_flat[ecore, slot] = np.asarray(edge_vals, np.float32)[eidx]
    sdst_flat[ecore, slot] = ed[eidx].astype(np.int16)

    gidx_rep = rep_idx(gidx_flat)                       # [C,128,TOT*8]
    sidx_rep = rep_idx(sdst_flat)
    vals_arr = val_flat.reshape(C, TOT, 128).transpose(0, 2, 1).copy()

    feat_pad = np.zeros((C, 512, R), np.float32)
    for c in range(C):
        feat_pad[c, :, :R_REAL] = features[order[np.arange(R_REAL) * C + c]].T

    b1r = np.asarray(b1, np.float32).reshape(2, 128).T.copy()
    b2r = np.broadcast_to(np.asarray(b2, np.float32), (128, 64)).copy()

    in_maps = []
    for c in range(C):
        in_maps.append({
            "feat": feat_pad[c],
            "W1": np.asarray(W1, np.float32),
            "b1r": b1r,
            "W2": np.asarray(W2, np.float32),
            "b2r": b2r,
            "gidx": gidx_rep[c],
            "sidx": sidx_rep[c],
            "vals": vals_arr[c],
            "zin": np.zeros((ACC_ROWS, 64), np.float32),
        })
    return in_maps, blocks, TOT, order


def build(blocks, TOT):
    nc = bacc.Bacc(None, num_devices=C, debug=False, target_bir_lowering=False)
    f32 = mybir.dt.float32
    i16 = mybir.dt.int16
    AG = mybir.AluOpType

    feat = nc.dram_tensor("feat", [512, R], f32, kind="ExternalInput")
    W1 = nc.dram_tensor("W1", [512, 256], f32, kind="ExternalInput")
    b1r = nc.dram_tensor("b1r", [128, 2], f32, kind="ExternalInput")
    W2 = nc.dram_tensor("W2", [256, 64], f32, kind="ExternalInput")
    b2r = nc.dram_tensor("b2r", [128, 64], f32, kind="ExternalInput")
    gidx = nc.dram_tensor("gidx", [128, TOT * 8], i16, kind="ExternalInput")
    sidx = nc.dram_tensor("sidx", [128, TOT * 8], i16, kind="ExternalInput")
    vals = nc.dram_tensor("vals", [128, TOT], f32, kind="ExternalInput")
    zin = nc.dram_tensor("zin", [ACC_ROWS, 64], f32, kind="ExternalInput")
    out = nc.dram_tensor("out", [R, 64], f32, kind="ExternalOutput")

    x_self = nc.dram_tensor("x_self", [R, 64], f32)
    x_full = nc.dram_tensor("x_full", [NTOT, 64], f32)
    acc = nc.dram_tensor("acc", [ACC_ROWS, 64], f32)

    x_self_v = x_self.ap().rearrange("(j p) c -> p j c", p=128)
    acc_x_v = acc.ap()[:R, :].rearrange("(j p) c -> p j c", p=128)
    out_v = out.ap().rearrange("(j p) c -> p j c", p=128)

    with tile.TileContext(nc) as tc:
        with (
            tc.tile_pool(name="const", bufs=1) as constp,
            tc.tile_pool(name="mlp", bufs=3) as mlpp,
            tc.tile_pool(name="ps", bufs=2, space="PSUM") as psump,
            tc.tile_pool(name="msg", bufs=3) as msgp,
            tc.tile_pool(name="idxp", bufs=3) as idxp,
            tc.tile_pool(name="cmb", bufs=1) as cmbp,
        ):
            w1_sb = constp.tile([128, 4, 256], f32)
            w2_sb = constp.tile([128, 2, 64], f32)
            b1_sb = constp.tile([128, 2], f32)
            b2_sb = constp.tile([128, 64], f32)
            vals_sb = constp.tile([128, TOT], f32)
            h01_sb = constp.tile([128, JR, 64], f32)
            x_sb = constp.tile([128, JR, 64], f32)

            nc.sync.dma_start(w1_sb[:], W1.ap().rearrange("(a p) h -> p a h", p=128))
            nc.sync.dma_start(w2_sb[:], W2.ap().rearrange("(a p) h -> p a h", p=128))
            nc.sync.dma_start(b1_sb[:], b1r.ap())
            nc.sync.dma_start(b2_sb[:], b2r.ap())
            nc.sync.dma_start(vals_sb[:], vals.ap())

            # ---- MLP ----
            for rb in range(R // 256):  # 49 blocks of 256 rows
                ft = mlpp.tile([128, 4, 256], f32, tag="ft")
                for it in range(4):
                    nc.sync.dma_start(
                        ft[:, it, :],
                        feat.ap()[it * 128:(it + 1) * 128, rb * 256:(rb + 1) * 256],
                    )
                x1t = mlpp.tile([128, 2, 256], f32, tag="x1t")
                for ht in range(2):
                    ps1 = psump.tile([128, 256], f32, tag="ps1")
                    for it in range(4):
                        nc.tensor.matmul(
                            ps1[:],
                            w1_sb[:, it, ht * 128:(ht + 1) * 128],
                            ft[:, it, :],
                            start=(it == 0),
                            stop=(it == 3),
                        )
                    nc.scalar.activation(
                        x1t[:, ht, :], ps1[:],
                        mybir.ActivationFunctionType.Relu,
                        bias=b1_sb[:, ht:ht + 1],
                    )
                for rt in range(2):
                    ps2 = psump.tile([128, 64], f32, tag="ps2")
                    for ht in range(2):
                        nc.tensor.matmul(
                            ps2[:],
                            x1t[:, ht, rt * 128:(rt + 1) * 128],
                            w2_sb[:, ht, :],
                            start=(ht == 0),
                            stop=(ht == 1),
                        )
                    j = rb * 2 + rt
                    nc.vector.tensor_tensor(
                        out=x_sb[:, j, :], in0=ps2[:], in1=b2_sb[:],
                        op=AG.add,
                    )
            nc.vector.tensor_scalar_mul(h01_sb[:], x_sb[:], ALPHA)
            nc.sync.dma_start(x_self_v, x_sb[:])

            # ---- propagation ----
            for k in range(K):
                nc.sync.dma_start(acc.ap()[:, :], zin.ap()[:, :])
                nc.gpsimd.collective_compute(
                    "AllGather",
                    AG.bypass,
                    replica_groups=[list(range(C))],
                    ins=[x_self.ap().opt()],
                    outs=[x_full.ap().opt()],
                )
                for (c, col0, ncol) in blocks:
                    n = ncol * 128
                    gi = idxp.tile([128, SUBC * 8], i16, tag="gi")
                    si = idxp.tile([128, SUBC * 8], i16, tag="si")
                    nc.sync.dma_start(gi[:, :ncol * 8], gidx.ap()[:, col0 * 8:(col0 + ncol) * 8])
                    nc.sync.dma_start(si[:, :ncol * 8], sidx.ap()[:, col0 * 8:(col0 + ncol) * 8])
                    mt = msgp.tile([128, SUBC, 64], f32, tag="mt")
                    nc.gpsimd.dma_gather(
                        mt[:, :ncol, :],
                        x_full.ap()[c * CHUNK:(c + 1) * CHUNK, :],
                        gi[:, :ncol * 8],
                        n, n, 64,
                        single_packet=False,
                    )
                    vb = vals_sb[:, col0:col0 + ncol].unsqueeze(2).to_broadcast((128, ncol, 64))
                    nc.vector.tensor_tensor(
                        out=mt[:, :ncol, :], in0=mt[:, :ncol, :], in1=vb,
                        op=AG.mult,
                    )
                    nc.gpsimd.dma_scatter_add(
                        acc.ap()[:, :],
                        mt[:, :ncol, :],
                        si[:, :ncol * 8],
                        n, n, 64,
                        single_packet=False,
                    )
                at = cmbp.tile([128, JR, 64], f32, tag="big")
                nc.sync.dma_start(at[:], acc_x_v)
                nc.vector.tensor_scalar_mul(x_sb[:], at[:], 1.0 - ALPHA)
                nc.vector.tensor_tensor(out=x_sb[:], in0=x_sb[:], in1=h01_sb[:], op=AG.add)
                if k < K - 1:
                    nc.sync.dma_start(x_self_v, x_sb[:])

            # ---- log_softmax ----
            mx = cmbp.tile([128, JR], f32, tag="mx")
            nc.vector.tensor_reduce(mx[:], x_sb[:], axis=mybir.AxisListType.X, op=AG.max)
            nc.vector.tensor_tensor(
                out=x_sb[:], in0=x_sb[:],
                in1=mx[:].unsqueeze(2).to_broadcast((128, JR, 64)),
                op=AG.subtract,
            )
            ex = cmbp.tile([128, JR, 64], f32, tag="big")
            nc.scalar.activation(ex[:], x_sb[:], mybir.ActivationFunctionType.Exp)
            sm = cmbp.tile([128, JR], f32, tag="sm")
            nc.vector.tensor_reduce(sm[:], ex[:], axis=mybir.AxisListType.X, op=AG.add)
            ls = cmbp.tile([128, JR], f32, tag="ls")
            nc.scalar.activation(ls[:], sm[:], mybir.ActivationFunctionType.Ln)
            nc.vector.tensor_tensor(
                out=x_sb[:], in0=x_sb[:],
                in1=ls[:].unsqueeze(2).to_broadcast((128, JR, 64)),
                op=AG.subtract,
            )
            nc.sync.dma_start(out_v, x_sb[:])

    if not nc.is_finalized():
        nc.finalize()
    return nc


_CACHE = {}


def kernel(features, W1, b1, W2, b2, edge_vals, edge_row, edge_col,
           trace=False, want_result=True):
    in_maps, blocks, TOT, order = prep(
        features, W1, b1, W2, b2, edge_vals, edge_row, edge_col
    )
    key = (TOT, len(blocks))
    if key not in _CACHE:
        _CACHE[key] = build(blocks, TOT)
    nc = _CACHE[key]
    res = bass_utils.run_bass_kernel_spmd(
        nc, in_maps, core_ids=list(range(C)), trace=trace
    )
    outs = [res.results[c]["out"] for c in range(C)]
    full = np.empty((N_NODES, 64), np.float32)
    i = np.arange(N_NODES)
    stacked = np.stack(outs)  # [C, R, 64]
    full[order] = stacked[i % C, i // C]
    if want_result:
        return full, res
    return full
